# revision 1
# baseline (speedup 1.0000x reference)
"""Trainium2 Bass kernel for nn_CausalVideoAttention (b=2, s=2048, d=512, 8 heads).

Sharding: 8 cores = (batch, head-pair): core c -> batch c//4, heads {2*(c%4), 2*(c%4)+1}.
Each core computes qkv projection for its head pair, causal attention over the full
sequence, and a partial output projection (its heads' slice of Wo), producing
out_core = (z_pair @ Wo_pair^T)^T  as a [512, 2048] tensor. Host sums the 4 cores of
each batch and transposes. No device collectives needed.

On-chip layout is "transposed domain": activations stored [feature, seq] so every
matmul has contraction on partitions and free dim 512 (full-rate fp32r).
"""

import sys

for _p in ("/opt/trn_rl_repo",):
    if _p not in sys.path:
        sys.path.insert(0, _p)

import numpy as np
import concourse.bass as bass
import concourse.mybir as mybir
import concourse.tile as tile
from concourse import bacc
from concourse.bass_utils import run_bass_kernel_spmd
from concourse.dve_ops import (
    RECIPROCAL_APPROX_FAST,
    RECIPROCAL_APPROX_NR,
    RECIP_APPROX_FAST_CONSTS,
)

F32 = mybir.dt.float32
F32R = mybir.dt.float32r
AF = mybir.ActivationFunctionType

B, S, D = 2, 2048, 512
NH, DH = 8, 64
P = 128          # partitions / tile edge
NT = S // P      # 16 s-tiles
NCH = D // P     # 4 din chunks
QC = 512         # q-chunk width
NQC = S // QC    # 4 q-chunks
EPS = 1e-6
NEG = -30000.0

_CACHE = {}


def _build_program():
    nc = bacc.Bacc("TRN2", target_bir_lowering=False, debug=False, num_devices=8)
    xb = nc.dram_tensor("xb", [S, D], F32R, kind="ExternalInput").ap()
    wq = nc.dram_tensor("wq", [P, D], F32R, kind="ExternalInput").ap()
    wk = nc.dram_tensor("wk", [P, D], F32R, kind="ExternalInput").ap()
    wv = nc.dram_tensor("wv", [P, D], F32R, kind="ExternalInput").ap()
    wo = nc.dram_tensor("wo", [D, P], F32R, kind="ExternalInput").ap()
    idn = nc.dram_tensor("idn", [P, P], F32R, kind="ExternalInput").ap()
    out = nc.dram_tensor("out", [D, S], F32, kind="ExternalOutput").ap()

    with tile.TileContext(nc) as tc:
        with tc.tile_pool(name="const", bufs=1) as cpool, \
             tc.tile_pool(name="big", bufs=1) as big:
            # ---- constants ----
            ident = cpool.tile([P, P], F32R, tag="ident")
            nc.sync.dma_start(ident[:], idn[:])
            tri1 = cpool.tile([P, P], F32, tag="tri1")
            nc.vector.memset(tri1[:], 0.0)
            # keep where qf - p >= 0 (strict upper-left of diag masked)
            nc.gpsimd.affine_select(out=tri1[:], in_=tri1[:],
                                    compare_op=mybir.AluOpType.is_ge, fill=NEG,
                                    base=0, channel_multiplier=-1, pattern=[[1, P]])
            tri2 = cpool.tile([P, 256], F32, tag="tri2")
            nc.vector.memset(tri2[:], 0.0)
            # keep where qf - p - 128 >= 0 (first 128 cols fully masked + triangle)
            nc.gpsimd.affine_select(out=tri2[:], in_=tri2[:],
                                    compare_op=mybir.AluOpType.is_ge, fill=NEG,
                                    base=-P, channel_multiplier=-1, pattern=[[1, 256]])
            sel = cpool.tile([P, 2], F32, tag="sel")
            nc.vector.memset(sel[:], 0.0)
            nc.vector.memset(sel[0:64, 0:1], 1.0)
            nc.vector.memset(sel[64:128, 1:2], 1.0)

            # ---- persistent big tiles ----
            xT = [big.tile([P, S], F32R, name=f"xT{c}", tag=f"xT{c}") for c in range(NCH)]
            qTr = big.tile([P, S], F32, tag="qTr")      # raw q^T (pre-norm)
            qTn = big.tile([P, S], F32R, tag="qTn")     # normalized q^T
            kT = big.tile([P, S], F32R, tag="kT")
            vT = big.tile([P, S], F32R, tag="vT")
            vaug0 = big.tile([P, 65 * NT], F32R, tag="vaug0")
            vaug1 = big.tile([P, P * NT], F32R, tag="vaug1")
            wqT = big.tile([P, D], F32R, tag="wqT")
            wkT = big.tile([P, D], F32R, tag="wkT")
            wvT = big.tile([P, D], F32R, tag="wvT")
            woT = big.tile([P, D], F32R, tag="woT")
            sqq = big.tile([P, S], F32, tag="sqq")
            sqk = big.tile([P, S], F32, tag="sqk")
            fqT = [big.tile([1, S], F32, name=f"fqT{h}", tag=f"fqT{h}") for h in range(2)]
            fqb = [big.tile([P, S], F32, name=f"fqb{h}", tag=f"fqb{h}") for h in range(2)]
            frec = big.tile([P, 64], F32, tag="frec")

            # ================= phase 1: load & transpose =================
            with tc.tile_pool(name="xin", bufs=3) as xin, \
                 tc.tile_pool(name="tps", bufs=4, space="PSUM") as tps:
                # weights in, transpose to W^T chunks
                for w_in, wT in ((wq, wqT), (wk, wkT), (wv, wvT)):
                    wsb = xin.tile([P, D], F32R, tag="wsb")
                    nc.sync.dma_start(wsb[:], w_in[:])
                    for c in range(NCH):
                        pt = tps.tile([P, P], F32R, tag="tp")
                        nc.tensor.transpose(pt[:], wsb[:, P * c:P * (c + 1)], ident[:])
                        nc.vector.tensor_copy(wT[:, P * c:P * (c + 1)], pt[:])
                # wo arrives [512,128]; view as [128, (c,128)] then transpose chunks
                wosb = xin.tile([P, D], F32R, tag="wsb")
                nc.sync.dma_start(wosb[:].rearrange("p (c j) -> p c j", c=NCH),
                                  wo.rearrange("(c p) j -> p c j", p=P))
                for c in range(NCH):
                    pt = tps.tile([P, P], F32R, tag="tp")
                    nc.tensor.transpose(pt[:], wosb[:, P * c:P * (c + 1)], ident[:])
                    nc.vector.tensor_copy(woT[:, P * c:P * (c + 1)], pt[:])

                for t in range(NT):
                    xsb = xin.tile([P, D], F32R, tag="xsb")
                    nc.sync.dma_start(xsb[:], xb[P * t:P * (t + 1), :])
                    for c in range(NCH):
                        pt = tps.tile([P, P], F32R, tag="tp")
                        nc.tensor.transpose(pt[:], xsb[:, P * c:P * (c + 1)], ident[:])
                        dst = xT[c][:, P * t:P * (t + 1)]
                        if c >= 2:
                            nc.scalar.copy(dst, pt[:])
                        else:
                            nc.vector.tensor_copy(dst, pt[:])

            # ================= phase 2: projections + rmsnorm =================
            with tc.tile_pool(name="pps", bufs=3, space="PSUM") as pps, \
                 tc.tile_pool(name="ssps", bufs=1, space="PSUM") as ssps, \
                 tc.tile_pool(name="fps", bufs=1, space="PSUM") as fps, \
                 tc.tile_pool(name="vps", bufs=2, space="PSUM") as vps:
                ss = ssps.tile([P, 64], F32, tag="ss")
                for sc in range(NQC):
                    qs = slice(QC * sc, QC * (sc + 1))
                    for name, wT, in2 in (("q", wqT, None), ("k", wkT, None), ("v", wvT, None)):
                        ps = pps.tile([P, QC], F32, tag="proj")
                        for c in range(NCH):
                            nc.tensor.matmul(ps[:], wT[:, P * c:P * (c + 1)],
                                             xT[c][:, qs], start=(c == 0), stop=(c == 3))
                        if name == "q":
                            nc.vector.tensor_copy(qTr[:, qs], ps[:])
                            nc.scalar.square(sqq[:, qs], ps[:])
                        elif name == "k":
                            nc.vector.tensor_copy(kT[:, qs], ps[:])
                            nc.scalar.square(sqk[:, qs], ps[:])
                        else:
                            nc.vector.tensor_copy(vT[:, qs], ps[:])
                    # sum-of-squares per head via selector matmul (plain fp32, exact)
                    for tl in range(4):
                        t = 4 * sc + tl
                        nc.tensor.matmul(ss[:, 2 * t:2 * t + 2],
                                         sqq[:, P * t:P * (t + 1)], sel[:],
                                         start=True, stop=True)
                        nc.tensor.matmul(ss[:, 32 + 2 * t:32 + 2 * t + 2],
                                         sqk[:, P * t:P * (t + 1)], sel[:],
                                         start=True, stop=True)

                # factors per chunk: 1 / (sqrt(ss/64) + eps)  [s-native orientation]
                for sc in range(NQC):
                    qs = slice(QC * sc, QC * (sc + 1))
                    for base in (0, 32):
                        cs = slice(base + 8 * sc, base + 8 * sc + 8)
                        nc.scalar.activation(frec[:, cs], ss[:, cs], AF.Sqrt,
                                             bias=0.0, scale=1.0 / DH)
                        nc.vector.tensor_scalar_add(frec[:, cs], frec[:, cs], EPS)
                        nc.vector.reciprocal(frec[:, cs], frec[:, cs])
                    for tl in range(4):
                        t = 4 * sc + tl
                        for h in range(2):
                            fp = fps.tile([1, P], F32, tag="fp")
                            nc.tensor.transpose(fp[:], frec[:, 2 * t + h:2 * t + h + 1],
                                                ident[:].bitcast(F32))
                            nc.vector.tensor_copy(fqT[h][:, P * t:P * (t + 1)], fp[:])
                    for h in range(2):
                        nc.gpsimd.partition_broadcast(fqb[h][:, qs], fqT[h][:, qs])
                    nc.vector.tensor_mul(qTn[0:64, qs], qTr[0:64, qs], fqb[0][0:64, qs])
                    nc.vector.tensor_mul(qTn[64:128, qs], qTr[64:128, qs], fqb[1][64:128, qs])

                # v back to native orientation, augmented with ones column
                nc.vector.memset(vaug0[:, 64::65].bitcast(F32), 1.0)
                nc.vector.memset(vaug1[:].bitcast(F32), 0.0)
                nc.vector.memset(vaug1[:, 0::P].bitcast(F32), 1.0)
                for t in range(NT):
                    vp = vps.tile([P, P], F32R, tag="vp")
                    nc.tensor.transpose(vp[:], vT[:, P * t:P * (t + 1)], ident[:])
                    nc.vector.tensor_copy(vaug0[:, 65 * t:65 * t + 64], vp[:, 0:64])
                    nc.vector.tensor_copy(vaug1[:, P * t + 64:P * t + 128], vp[:, 64:128])

            # ================= phase 3: attention + output =================
            with tc.tile_pool(name="scps", bufs=4, space="PSUM") as scps, \
                 tc.tile_pool(name="ztps", bufs=2, space="PSUM") as ztps, \
                 tc.tile_pool(name="ops", bufs=2, space="PSUM") as ops, \
                 tc.tile_pool(name="att", bufs=6) as att, \
                 tc.tile_pool(name="nrm", bufs=2) as nrm:
                C = RECIP_APPROX_FAST_CONSTS
                for j in range(NQC):
                    q0 = QC * j
                    zTn = nrm.tile([P, QC], F32R, tag="zTn")
                    nkb = 4 * j + 4
                    for h in range(2):
                        hs = slice(64 * h, 64 * (h + 1))
                        zt = ztps.tile([P, QC], F32, tag="zt")
                        for kb in range(nkb):
                            i = kb - 4 * j
                            qoff = 0
                            if i >= 0:
                                qoff = 256 if i == 3 else P * i
                            fr = QC - qoff
                            sc_ps = scps.tile([P, fr], F32, tag="sc")
                            nc.tensor.matmul(sc_ps[:],
                                             kT[hs, P * kb:P * (kb + 1)],
                                             qTn[hs, q0 + qoff:q0 + QC],
                                             start=True, stop=True)
                            if i == 3:
                                nc.vector.tensor_add(sc_ps[:, 0:256], sc_ps[:, 0:256], tri2[:])
                            elif i >= 0:
                                nc.vector.tensor_add(sc_ps[:, 0:P], sc_ps[:, 0:P], tri1[:])
                            eT = att.tile([P, fr], F32R, tag="eT")
                            nc.scalar.activation(eT[:], sc_ps[:], AF.Exp, bias=0.0,
                                                 scale=frec[:, 32 + 2 * kb + h:33 + 2 * kb + h])
                            if h == 0:
                                nc.tensor.matmul(zt[0:65, qoff:QC],
                                                 vaug0[:, 65 * kb:65 * kb + 65], eT[:],
                                                 start=(kb == 0), stop=(kb == nkb - 1))
                            else:
                                nc.tensor.matmul(zt[:, qoff:QC],
                                                 vaug1[:, P * kb:P * (kb + 1)], eT[:],
                                                 start=(kb == 0), stop=(kb == nkb - 1))
                        # softmax denominators: h0 row at p64, h1 row at p0
                        dp = 64 if h == 0 else 0
                        ds = slice(dp, dp + 1)
                        dra = nrm.tile([P, QC], F32, tag="dra")
                        drb = nrm.tile([P, QC], F32, tag="drb")
                        nc.vector.tensor_copy(dra[ds, :], zt[ds, :])
                        nc.vector.reciprocal(drb[ds, :], dra[ds, :])
                        if h == 0:
                            d0 = nrm.tile([1, QC], F32, tag="d0")
                            nc.sync.dma_start(d0[:], drb[64:65, :])
                            src_row = d0[:]
                        else:
                            src_row = drb[0:1, :]
                        db = nrm.tile([P, QC], F32, tag="db")
                        nc.gpsimd.partition_broadcast(db[:], src_row)
                        zrows = zt[0:64, :] if h == 0 else zt[64:128, :]
                        nc.vector.tensor_mul(zTn[hs, :], zrows, db[hs, :])
                    for dc in range(NCH):
                        op = ops.tile([P, QC], F32, tag="op")
                        nc.tensor.matmul(op[:], woT[:, P * dc:P * (dc + 1)], zTn[:],
                                         start=True, stop=True)
                        osb = att.tile([P, QC], F32, tag="osb")
                        if dc % 2 == 0:
                            nc.vector.tensor_copy(osb[:], op[:])
                        else:
                            nc.scalar.copy(osb[:], op[:])
                        nc.sync.dma_start(out[P * dc:P * (dc + 1), q0:q0 + QC], osb[:])

    nc.finalize()
    return nc


def _numpy_reference(x, Wqkv, Wo, scale_q, scale_k, mask):
    b, s, d = x.shape
    dh = d // NH
    qkv = x @ Wqkv.T
    q, k, v = np.split(qkv, 3, axis=-1)

    def rms(t, scale):
        r = np.sqrt(np.mean(np.square(t), axis=-1, keepdims=True)) + EPS
        return t / r * scale

    q = rms(q.reshape(b, s, NH, dh), scale_q)
    k = rms(k.reshape(b, s, NH, dh), scale_k)
    v = v.reshape(b, s, NH, dh)
    attn = np.einsum('bqhd,bkhd->bhqk', q, k)
    attn = np.where(mask[None, None], -np.inf, attn)
    attn = attn - attn.max(axis=-1, keepdims=True)
    p = np.exp(attn)
    p = p / p.sum(axis=-1, keepdims=True)
    z = np.einsum('bhqk,bkhd->bqhd', p, v).reshape(b, s, d)
    return (z @ Wo.T).astype(np.float32)


def kernel(x, Wqkv, Wo, scale_q, scale_k, mask):
    x = np.asarray(x, np.float32)
    Wqkv = np.asarray(Wqkv, np.float32)
    Wo = np.asarray(Wo, np.float32)
    causal = np.triu(np.ones((S, S), dtype=bool), k=1)
    if (not np.allclose(np.asarray(scale_q), 1.0) or
            not np.allclose(np.asarray(scale_k), 1.0) or
            not np.array_equal(np.asarray(mask), causal) or
            x.shape != (B, S, D)):
        return _numpy_reference(x, Wqkv, Wo, np.asarray(scale_q), np.asarray(scale_k),
                                np.asarray(mask))

    if "nc" not in _CACHE:
        _CACHE["nc"] = _build_program()
    nc = _CACHE["nc"]

    in_maps = []
    for core in range(8):
        bb = core // 4
        h0 = 2 * (core % 4)
        r = slice(DH * h0, DH * h0 + P)
        in_maps.append({
            "xb": np.ascontiguousarray(x[bb]),
            "wq": np.ascontiguousarray(Wqkv[r, :]),
            "wk": np.ascontiguousarray(Wqkv[D:][r, :]),
            "wv": np.ascontiguousarray(Wqkv[2 * D:][r, :]),
            "wo": np.ascontiguousarray(Wo[:, r]),
            "idn": np.eye(P, dtype=np.float32),
        })
    _CACHE["last_in_maps"] = in_maps
    res = run_bass_kernel_spmd(nc, in_maps, core_ids=list(range(8)))
    outp = np.zeros((B, S, D), np.float32)
    for core in range(8):
        outp[core // 4] += res.results[core]["out"].T
    return outp



# revision 42
# speedup vs baseline: 639.7724x; 639.7724x over previous
"""Trainium2 Bass kernel for nn_CausalVideoAttention (b=2, s=2048, d=512, 8 heads).

Sharding: 8 cores = (batch, head-pair): core c -> batch c//4, heads {2*(c%4), 2*(c%4)+1}.
Each core computes qkv projection for its head pair, causal attention over the full
sequence, and a partial output projection (its heads' slice of Wo), producing
out_core = (z_pair @ Wo_pair^T)^T  as a [512, 2048] tensor. Host sums the 4 cores of
each batch and transposes. No device collectives needed.

On-chip layout is "transposed domain": activations stored [feature, seq] so every
matmul has contraction on partitions and free dim 512 (full-rate fp32r).

`reps` wraps the whole body in a hardware For_i loop: used by test.py to measure
the true steady-state per-iteration execution time (transfers cancel in the
wall-clock slope).
"""

import sys
from contextlib import nullcontext

for _p in ("/opt/trn_rl_repo",):
    if _p not in sys.path:
        sys.path.insert(0, _p)

import numpy as np
import concourse.bass as bass
import concourse.mybir as mybir
import concourse.tile as tile
from concourse import bacc
from concourse.bass_utils import run_bass_kernel_spmd

F32 = mybir.dt.float32
F32R = mybir.dt.float32r
AF = mybir.ActivationFunctionType

B, S, D = 2, 2048, 512
NH, DH = 8, 64
P = 128          # partitions / tile edge
NT = S // P      # 16 s-tiles
NCH = D // P     # 4 din chunks
QC = 512         # q-chunk width
NQC = S // QC    # 4 q-chunks
EPS = 1e-6
NEG = -30000.0

_CACHE = {}


def _build_program(reps=1):
    nc = bacc.Bacc("TRN2", target_bir_lowering=False, debug=False, num_devices=8)
    xb = nc.dram_tensor("xb", [S, D], F32R, kind="ExternalInput").ap()
    wq = nc.dram_tensor("wq", [P, D], F32R, kind="ExternalInput").ap()
    wk = nc.dram_tensor("wk", [P, D], F32R, kind="ExternalInput").ap()
    wv = nc.dram_tensor("wv", [P, D], F32R, kind="ExternalInput").ap()
    wo = nc.dram_tensor("wo", [D, P], F32R, kind="ExternalInput").ap()
    idn = nc.dram_tensor("idn", [P, P], F32R, kind="ExternalInput").ap()
    out = nc.dram_tensor("out", [D, S], F32, kind="ExternalOutput").ap()

    with tile.TileContext(nc) as tc:
        with (tc.For_i(0, reps) if reps > 1 else nullcontext()), \
             tc.tile_pool(name="const", bufs=1) as cpool, \
             tc.tile_pool(name="big", bufs=1) as big:
            # ---- constants ----
            ident = cpool.tile([P, P], F32R, tag="ident")
            nc.sync.dma_start(ident[:], idn[:])
            tri1 = cpool.tile([P, P], F32, tag="tri1")
            nc.vector.memset(tri1[:], 0.0)
            # keep where qf - p >= 0 (strict upper-left of diag masked)
            nc.gpsimd.affine_select(out=tri1[:], in_=tri1[:],
                                    compare_op=mybir.AluOpType.is_ge, fill=NEG,
                                    base=0, channel_multiplier=-1, pattern=[[1, P]])
            tri2 = cpool.tile([P, 256], F32, tag="tri2")
            nc.vector.memset(tri2[:], 0.0)
            # keep where qf - p - 128 >= 0 (first 128 cols fully masked + triangle)
            nc.gpsimd.affine_select(out=tri2[:], in_=tri2[:],
                                    compare_op=mybir.AluOpType.is_ge, fill=NEG,
                                    base=-P, channel_multiplier=-1, pattern=[[1, 256]])
            sel = cpool.tile([P, 2], F32, tag="sel")
            nc.vector.memset(sel[:], 0.0)
            nc.vector.memset(sel[0:64, 0:1], 1.0)
            nc.vector.memset(sel[64:128, 1:2], 1.0)

            # ---- persistent big tiles ----
            xT = [big.tile([P, S], F32R, name=f"xT{c}", tag=f"xT{c}") for c in range(NCH)]
            qTr = big.tile([P, S], F32, tag="qTr")      # raw q^T (pre-norm)
            qTn = big.tile([P, S], F32R, tag="qTn")     # normalized q^T
            kT = big.tile([P, S], F32R, tag="kT")
            vT = big.tile([P, S], F32R, tag="vT")
            vaug0 = big.tile([P, 65 * NT], F32R, tag="vaug0")
            vaug1 = big.tile([P, P * NT], F32R, tag="vaug1")
            wqT = big.tile([P, D], F32R, tag="wqT")
            wkT = big.tile([P, D], F32R, tag="wkT")
            wvT = big.tile([P, D], F32R, tag="wvT")
            woT = big.tile([P, D], F32R, tag="woT")
            sqq = big.tile([P, S], F32, tag="sqq")
            sqk = big.tile([P, S], F32, tag="sqk")
            fqT = [big.tile([1, S], F32, name=f"fqT{h}", tag=f"fqT{h}") for h in range(2)]
            fqb = [big.tile([P, S], F32, name=f"fqb{h}", tag=f"fqb{h}") for h in range(2)]
            frec = big.tile([P, 64], F32, tag="frec")

            # ================= phase 1: load & transpose =================
            with tc.tile_pool(name="xin", bufs=3) as xin, \
                 tc.tile_pool(name="tps", bufs=4, space="PSUM") as tps:
                # weights in, transpose to W^T chunks
                for w_in, wT in ((wq, wqT), (wk, wkT), (wv, wvT)):
                    wsb = xin.tile([P, D], F32R, tag="wsb")
                    nc.sync.dma_start(wsb[:], w_in[:])
                    for c in range(NCH):
                        pt = tps.tile([P, P], F32R, tag="tp")
                        nc.tensor.transpose(pt[:], wsb[:, P * c:P * (c + 1)], ident[:])
                        nc.vector.tensor_copy(wT[:, P * c:P * (c + 1)], pt[:])
                # wo arrives [512,128]; view as [128, (c,128)] then transpose chunks
                wosb = xin.tile([P, D], F32R, tag="wsb")
                nc.sync.dma_start(wosb[:].rearrange("p (c j) -> p c j", c=NCH),
                                  wo.rearrange("(c p) j -> p c j", p=P))
                for c in range(NCH):
                    pt = tps.tile([P, P], F32R, tag="tp")
                    nc.tensor.transpose(pt[:], wosb[:, P * c:P * (c + 1)], ident[:])
                    nc.vector.tensor_copy(woT[:, P * c:P * (c + 1)], pt[:])

                for t in range(NT):
                    xsb = xin.tile([P, D], F32R, tag="xsb")
                    nc.sync.dma_start(xsb[:], xb[P * t:P * (t + 1), :])
                    for c in range(NCH):
                        pt = tps.tile([P, P], F32R, tag="tp")
                        nc.tensor.transpose(pt[:], xsb[:, P * c:P * (c + 1)], ident[:])
                        dst = xT[c][:, P * t:P * (t + 1)]
                        if c >= 2:
                            nc.scalar.copy(dst, pt[:])
                        else:
                            nc.vector.tensor_copy(dst, pt[:])

            # ================= phase 2: projections + rmsnorm =================
            with tc.tile_pool(name="pps", bufs=3, space="PSUM") as pps, \
                 tc.tile_pool(name="ssps", bufs=1, space="PSUM") as ssps, \
                 tc.tile_pool(name="fps", bufs=1, space="PSUM") as fps, \
                 tc.tile_pool(name="vps", bufs=2, space="PSUM") as vps:
                ss = ssps.tile([P, 64], F32, tag="ss")
                for sc in range(NQC):
                    qs = slice(QC * sc, QC * (sc + 1))
                    for name, wT, in2 in (("q", wqT, None), ("k", wkT, None), ("v", wvT, None)):
                        ps = pps.tile([P, QC], F32, tag="proj")
                        for c in range(NCH):
                            nc.tensor.matmul(ps[:], wT[:, P * c:P * (c + 1)],
                                             xT[c][:, qs], start=(c == 0), stop=(c == 3))
                        if name == "q":
                            nc.vector.tensor_copy(qTr[:, qs], ps[:])
                            nc.scalar.square(sqq[:, qs], ps[:])
                        elif name == "k":
                            nc.vector.tensor_copy(kT[:, qs], ps[:])
                            nc.scalar.square(sqk[:, qs], ps[:])
                        else:
                            nc.vector.tensor_copy(vT[:, qs], ps[:])
                    # sum-of-squares per head via selector matmul (plain fp32, exact)
                    for tl in range(4):
                        t = 4 * sc + tl
                        nc.tensor.matmul(ss[:, 2 * t:2 * t + 2],
                                         sqq[:, P * t:P * (t + 1)], sel[:],
                                         start=True, stop=True)
                        nc.tensor.matmul(ss[:, 32 + 2 * t:32 + 2 * t + 2],
                                         sqk[:, P * t:P * (t + 1)], sel[:],
                                         start=True, stop=True)

                # factors per chunk: 1 / (sqrt(ss/64) + eps)  [s-native orientation]
                for sc in range(NQC):
                    qs = slice(QC * sc, QC * (sc + 1))
                    for base in (0, 32):
                        cs = slice(base + 8 * sc, base + 8 * sc + 8)
                        nc.scalar.activation(frec[:, cs], ss[:, cs], AF.Sqrt,
                                             bias=0.0, scale=1.0 / DH)
                        nc.vector.tensor_scalar_add(frec[:, cs], frec[:, cs], EPS)
                        nc.vector.reciprocal(frec[:, cs], frec[:, cs])
                    for tl in range(4):
                        t = 4 * sc + tl
                        for h in range(2):
                            fp = fps.tile([1, P], F32, tag="fp")
                            nc.tensor.transpose(fp[:], frec[:, 2 * t + h:2 * t + h + 1],
                                                ident[:].bitcast(F32))
                            nc.vector.tensor_copy(fqT[h][:, P * t:P * (t + 1)], fp[:])
                    for h in range(2):
                        nc.gpsimd.partition_broadcast(fqb[h][:, qs], fqT[h][:, qs])
                    nc.vector.tensor_mul(qTn[0:64, qs], qTr[0:64, qs], fqb[0][0:64, qs])
                    nc.vector.tensor_mul(qTn[64:128, qs], qTr[64:128, qs], fqb[1][64:128, qs])

                # v back to native orientation, augmented with ones column
                nc.vector.memset(vaug0[:, 64::65].bitcast(F32), 1.0)
                nc.vector.memset(vaug1[:].bitcast(F32), 0.0)
                nc.vector.memset(vaug1[:, 0::P].bitcast(F32), 1.0)
                for t in range(NT):
                    vp = vps.tile([P, P], F32R, tag="vp")
                    nc.tensor.transpose(vp[:], vT[:, P * t:P * (t + 1)], ident[:])
                    nc.vector.tensor_copy(vaug0[:, 65 * t:65 * t + 64], vp[:, 0:64])
                    nc.vector.tensor_copy(vaug1[:, P * t + 64:P * t + 128], vp[:, 64:128])

            # ================= phase 3: attention + output =================
            with tc.tile_pool(name="scps", bufs=4, space="PSUM") as scps, \
                 tc.tile_pool(name="ztps", bufs=2, space="PSUM") as ztps, \
                 tc.tile_pool(name="ops", bufs=2, space="PSUM") as ops, \
                 tc.tile_pool(name="att", bufs=6) as att, \
                 tc.tile_pool(name="nrm", bufs=2) as nrm:
                for j in range(NQC):
                    q0 = QC * j
                    zTn = nrm.tile([P, QC], F32R, tag="zTn")
                    nkb = 4 * j + 4
                    for h in range(2):
                        hs = slice(64 * h, 64 * (h + 1))
                        zt = ztps.tile([P, QC], F32, tag="zt")
                        for kb in range(nkb):
                            i = kb - 4 * j
                            qoff = 0
                            if i >= 0:
                                qoff = 256 if i == 3 else P * i
                            fr = QC - qoff
                            sc_ps = scps.tile([P, fr], F32, tag="sc")
                            nc.tensor.matmul(sc_ps[:],
                                             kT[hs, P * kb:P * (kb + 1)],
                                             qTn[hs, q0 + qoff:q0 + QC],
                                             start=True, stop=True)
                            if i == 3:
                                nc.vector.tensor_add(sc_ps[:, 0:256], sc_ps[:, 0:256], tri2[:])
                            elif i >= 0:
                                nc.vector.tensor_add(sc_ps[:, 0:P], sc_ps[:, 0:P], tri1[:])
                            eT = att.tile([P, fr], F32R, tag="eT")
                            nc.scalar.activation(eT[:], sc_ps[:], AF.Exp, bias=0.0,
                                                 scale=frec[:, 32 + 2 * kb + h:33 + 2 * kb + h])
                            if h == 0:
                                nc.tensor.matmul(zt[0:65, qoff:QC],
                                                 vaug0[:, 65 * kb:65 * kb + 65], eT[:],
                                                 start=(kb == 0), stop=(kb == nkb - 1))
                            else:
                                nc.tensor.matmul(zt[:, qoff:QC],
                                                 vaug1[:, P * kb:P * (kb + 1)], eT[:],
                                                 start=(kb == 0), stop=(kb == nkb - 1))
                        # softmax denominators: h0 row at p64, h1 row at p0
                        dp = 64 if h == 0 else 0
                        ds = slice(dp, dp + 1)
                        dra = nrm.tile([P, QC], F32, tag="dra")
                        drb = nrm.tile([P, QC], F32, tag="drb")
                        nc.vector.tensor_copy(dra[ds, :], zt[ds, :])
                        nc.vector.reciprocal(drb[ds, :], dra[ds, :])
                        if h == 0:
                            d0 = nrm.tile([1, QC], F32, tag="d0")
                            nc.sync.dma_start(d0[:], drb[64:65, :])
                            src_row = d0[:]
                        else:
                            src_row = drb[0:1, :]
                        db = nrm.tile([P, QC], F32, tag="db")
                        nc.gpsimd.partition_broadcast(db[:], src_row)
                        zrows = zt[0:64, :] if h == 0 else zt[64:128, :]
                        nc.vector.tensor_mul(zTn[hs, :], zrows, db[hs, :])
                    for dc in range(NCH):
                        op = ops.tile([P, QC], F32, tag="op")
                        nc.tensor.matmul(op[:], woT[:, P * dc:P * (dc + 1)], zTn[:],
                                         start=True, stop=True)
                        osb = att.tile([P, QC], F32, tag="osb")
                        if dc % 2 == 0:
                            nc.vector.tensor_copy(osb[:], op[:])
                        else:
                            nc.scalar.copy(osb[:], op[:])
                        nc.sync.dma_start(out[P * dc:P * (dc + 1), q0:q0 + QC], osb[:])

    nc.finalize()
    return nc


def _numpy_reference(x, Wqkv, Wo, scale_q, scale_k, mask):
    b, s, d = x.shape
    dh = d // NH
    qkv = x @ Wqkv.T
    q, k, v = np.split(qkv, 3, axis=-1)

    def rms(t, scale):
        r = np.sqrt(np.mean(np.square(t), axis=-1, keepdims=True)) + EPS
        return t / r * scale

    q = rms(q.reshape(b, s, NH, dh), scale_q)
    k = rms(k.reshape(b, s, NH, dh), scale_k)
    v = v.reshape(b, s, NH, dh)
    attn = np.einsum('bqhd,bkhd->bhqk', q, k)
    attn = np.where(mask[None, None], -np.inf, attn)
    attn = attn - attn.max(axis=-1, keepdims=True)
    p = np.exp(attn)
    p = p / p.sum(axis=-1, keepdims=True)
    z = np.einsum('bhqk,bkhd->bqhd', p, v).reshape(b, s, d)
    return (z @ Wo.T).astype(np.float32)


def kernel(x, Wqkv, Wo, scale_q, scale_k, mask):
    x = np.asarray(x, np.float32)
    Wqkv = np.asarray(Wqkv, np.float32)
    Wo = np.asarray(Wo, np.float32)
    causal = np.triu(np.ones((S, S), dtype=bool), k=1)
    if (not np.allclose(np.asarray(scale_q), 1.0) or
            not np.allclose(np.asarray(scale_k), 1.0) or
            not np.array_equal(np.asarray(mask), causal) or
            x.shape != (B, S, D)):
        return _numpy_reference(x, Wqkv, Wo, np.asarray(scale_q), np.asarray(scale_k),
                                np.asarray(mask))

    if "nc" not in _CACHE:
        _CACHE["nc"] = _build_program()
    nc = _CACHE["nc"]

    in_maps = []
    for core in range(8):
        bb = core // 4
        h0 = 2 * (core % 4)
        r = slice(DH * h0, DH * h0 + P)
        in_maps.append({
            "xb": np.ascontiguousarray(x[bb]),
            "wq": np.ascontiguousarray(Wqkv[r, :]),
            "wk": np.ascontiguousarray(Wqkv[D:][r, :]),
            "wv": np.ascontiguousarray(Wqkv[2 * D:][r, :]),
            "wo": np.ascontiguousarray(Wo[:, r]),
            "idn": np.eye(P, dtype=np.float32),
        })
    _CACHE["last_in_maps"] = in_maps
    res = run_bass_kernel_spmd(nc, in_maps, core_ids=list(range(8)))
    outp = np.zeros((B, S, D), np.float32)
    for core in range(8):
        outp[core // 4] += res.results[core]["out"].T
    return outp


# revision 44
# speedup vs baseline: 761.6078x; 1.1904x over previous
"""Trainium2 Bass kernel for nn_CausalVideoAttention (b=2, s=2048, d=512, 8 heads).

Sharding: 8 cores = (batch, head-pair): core c -> batch c//4, heads {2*(c%4), 2*(c%4)+1}.
Each core computes qkv projection for its head pair, causal attention over the full
sequence, and a partial output projection (its heads' slice of Wo), producing
out_core = (z_pair @ Wo_pair^T)^T  as a [512, 2048] tensor. Host sums the 4 cores of
each batch and transposes. No device collectives needed.

On-chip layout is "transposed domain": activations stored [feature, seq] so every
matmul has contraction on partitions and free dim 512 (full-rate fp32r).

`reps` wraps the whole body in a hardware For_i loop: used by test.py to measure
the true steady-state per-iteration execution time (transfers cancel in the
wall-clock slope).
"""

import sys
from contextlib import nullcontext

for _p in ("/opt/trn_rl_repo",):
    if _p not in sys.path:
        sys.path.insert(0, _p)

import numpy as np
import concourse.bass as bass
import concourse.mybir as mybir
import concourse.tile as tile
from concourse import bacc
from concourse.bass_utils import run_bass_kernel_spmd

F32 = mybir.dt.float32
F32R = mybir.dt.float32r
AF = mybir.ActivationFunctionType

B, S, D = 2, 2048, 512
NH, DH = 8, 64
P = 128          # partitions / tile edge
NT = S // P      # 16 s-tiles
NCH = D // P     # 4 din chunks
QC = 512         # q-chunk width
NQC = S // QC    # 4 q-chunks
EPS = 1e-6
NEG = -30000.0

_CACHE = {}


def _build_program(reps=1):
    nc = bacc.Bacc("TRN2", target_bir_lowering=False, debug=False, num_devices=8)
    xb = nc.dram_tensor("xb", [S, D], F32R, kind="ExternalInput").ap()
    wt4 = nc.dram_tensor("wt4", [P, 4 * D], F32R, kind="ExternalInput").ap()
    idn = nc.dram_tensor("idn", [P, P], F32R, kind="ExternalInput").ap()
    out = nc.dram_tensor("out", [D, S], F32, kind="ExternalOutput").ap()

    with tile.TileContext(nc) as tc:
        with (tc.For_i(0, reps) if reps > 1 else nullcontext()), \
             tc.tile_pool(name="const", bufs=1) as cpool, \
             tc.tile_pool(name="big", bufs=1) as big:
            # ---- constants ----
            ident = cpool.tile([P, P], F32R, tag="ident")
            nc.sync.dma_start(ident[:], idn[:])
            tri1 = cpool.tile([P, P], F32, tag="tri1")
            nc.vector.memset(tri1[:], 0.0)
            # keep where qf - p >= 0 (strict upper-left of diag masked)
            nc.gpsimd.affine_select(out=tri1[:], in_=tri1[:],
                                    compare_op=mybir.AluOpType.is_ge, fill=NEG,
                                    base=0, channel_multiplier=-1, pattern=[[1, P]])
            tri2 = cpool.tile([P, 256], F32, tag="tri2")
            nc.vector.memset(tri2[:], 0.0)
            # keep where qf - p - 128 >= 0 (first 128 cols fully masked + triangle)
            nc.gpsimd.affine_select(out=tri2[:], in_=tri2[:],
                                    compare_op=mybir.AluOpType.is_ge, fill=NEG,
                                    base=-P, channel_multiplier=-1, pattern=[[1, 256]])
            sel = cpool.tile([P, 2], F32, tag="sel")
            nc.vector.memset(sel[:], 0.0)
            nc.vector.memset(sel[0:64, 0:1], 1.0)
            nc.vector.memset(sel[64:128, 1:2], 1.0)

            # ---- persistent big tiles ----
            xT = [big.tile([P, S], F32R, name=f"xT{c}", tag=f"xT{c}") for c in range(NCH)]
            qTr = big.tile([P, S], F32, tag="qTr")      # raw q^T (pre-norm)
            qTn = big.tile([P, S], F32R, tag="qTn")     # normalized q^T
            kT = big.tile([P, S], F32R, tag="kT")
            vT = big.tile([P, S], F32R, tag="vT")
            vaug0 = big.tile([P, 65 * NT], F32R, tag="vaug0")
            vaug1 = big.tile([P, P * NT], F32R, tag="vaug1")
            wt = big.tile([P, 4 * D], F32R, tag="wt")  # wqT|wkT|wvT|woT packed
            wqT = wt[:, 0 * D:1 * D]
            wkT = wt[:, 1 * D:2 * D]
            wvT = wt[:, 2 * D:3 * D]
            woT = wt[:, 3 * D:4 * D]
            sqq = big.tile([P, S], F32, tag="sqq")
            sqk = big.tile([P, S], F32, tag="sqk")
            fqT = [big.tile([1, S], F32, name=f"fqT{h}", tag=f"fqT{h}") for h in range(2)]
            fqb = [big.tile([P, S], F32, name=f"fqb{h}", tag=f"fqb{h}") for h in range(2)]
            frec = big.tile([P, 64], F32, tag="frec")

            # ================= phase 1: load & transpose =================
            with tc.tile_pool(name="xin", bufs=2) as xin, \
                 tc.tile_pool(name="tps", bufs=4, space="PSUM") as tps:
                # weights arrive host-pretransposed/packed: one DMA
                nc.sync.dma_start(wt[:], wt4[:])
                # x: one DMA per 4 s-tiles (first group per-tile for latency)
                for g in range(NT // 4):
                    xsb = xin.tile([P, 4 * D], F32R, tag="xsb")
                    if g == 0:
                        for tl in range(4):
                            nc.sync.dma_start(xsb[:, D * tl:D * (tl + 1)],
                                              xb[P * tl:P * (tl + 1), :])
                    else:
                        nc.sync.dma_start(
                            xsb[:].rearrange("p (t d) -> p t d", t=4),
                            xb.rearrange("(t p) d -> p t d", p=P)[:, 4 * g:4 * g + 4, :])
                    for tl in range(4):
                        t = 4 * g + tl
                        for c in range(NCH):
                            pt = tps.tile([P, P], F32R, tag="tp")
                            nc.tensor.transpose(pt[:],
                                                xsb[:, D * tl + P * c:D * tl + P * (c + 1)],
                                                ident[:])
                            dst = xT[c][:, P * t:P * (t + 1)]
                            if c >= 2:
                                nc.scalar.copy(dst, pt[:])
                            else:
                                nc.vector.tensor_copy(dst, pt[:])

            # ================= phase 2: projections + rmsnorm =================
            with tc.tile_pool(name="pps", bufs=3, space="PSUM") as pps, \
                 tc.tile_pool(name="ssps", bufs=1, space="PSUM") as ssps, \
                 tc.tile_pool(name="fps", bufs=1, space="PSUM") as fps, \
                 tc.tile_pool(name="vps", bufs=2, space="PSUM") as vps:
                ss = ssps.tile([P, 64], F32, tag="ss")
                for sc in range(NQC):
                    qs = slice(QC * sc, QC * (sc + 1))
                    for name, wT, in2 in (("q", wqT, None), ("k", wkT, None), ("v", wvT, None)):
                        ps = pps.tile([P, QC], F32, tag="proj")
                        for c in range(NCH):
                            nc.tensor.matmul(ps[:], wT[:, P * c:P * (c + 1)],
                                             xT[c][:, qs], start=(c == 0), stop=(c == 3))
                        if name == "q":
                            nc.vector.tensor_copy(qTr[:, qs], ps[:])
                            nc.scalar.square(sqq[:, qs], ps[:])
                        elif name == "k":
                            nc.vector.tensor_copy(kT[:, qs], ps[:])
                            nc.scalar.square(sqk[:, qs], ps[:])
                        else:
                            nc.vector.tensor_copy(vT[:, qs], ps[:])
                    # sum-of-squares per head via selector matmul (plain fp32, exact)
                    for tl in range(4):
                        t = 4 * sc + tl
                        nc.tensor.matmul(ss[:, 2 * t:2 * t + 2],
                                         sqq[:, P * t:P * (t + 1)], sel[:],
                                         start=True, stop=True)
                        nc.tensor.matmul(ss[:, 32 + 2 * t:32 + 2 * t + 2],
                                         sqk[:, P * t:P * (t + 1)], sel[:],
                                         start=True, stop=True)

                # factors per chunk: 1 / (sqrt(ss/64) + eps)  [s-native orientation]
                for sc in range(NQC):
                    qs = slice(QC * sc, QC * (sc + 1))
                    for base in (0, 32):
                        cs = slice(base + 8 * sc, base + 8 * sc + 8)
                        nc.scalar.activation(frec[:, cs], ss[:, cs], AF.Sqrt,
                                             bias=0.0, scale=1.0 / DH)
                        nc.vector.tensor_scalar_add(frec[:, cs], frec[:, cs], EPS)
                        nc.vector.reciprocal(frec[:, cs], frec[:, cs])
                    for tl in range(4):
                        t = 4 * sc + tl
                        for h in range(2):
                            fp = fps.tile([1, P], F32, tag="fp")
                            nc.tensor.transpose(fp[:], frec[:, 2 * t + h:2 * t + h + 1],
                                                ident[:].bitcast(F32))
                            nc.vector.tensor_copy(fqT[h][:, P * t:P * (t + 1)], fp[:])
                    for h in range(2):
                        nc.gpsimd.partition_broadcast(fqb[h][:, qs], fqT[h][:, qs])
                    nc.vector.tensor_mul(qTn[0:64, qs], qTr[0:64, qs], fqb[0][0:64, qs])
                    nc.vector.tensor_mul(qTn[64:128, qs], qTr[64:128, qs], fqb[1][64:128, qs])

                # v back to native orientation, augmented with ones column
                nc.vector.memset(vaug0[:, 64::65].bitcast(F32), 1.0)
                nc.vector.memset(vaug1[:].bitcast(F32), 0.0)
                nc.vector.memset(vaug1[:, 0::P].bitcast(F32), 1.0)
                for t in range(NT):
                    vp = vps.tile([P, P], F32R, tag="vp")
                    nc.tensor.transpose(vp[:], vT[:, P * t:P * (t + 1)], ident[:])
                    nc.vector.tensor_copy(vaug0[:, 65 * t:65 * t + 64], vp[:, 0:64])
                    nc.vector.tensor_copy(vaug1[:, P * t + 64:P * t + 128], vp[:, 64:128])

            # ================= phase 3: attention + output =================
            with tc.tile_pool(name="scps", bufs=4, space="PSUM") as scps, \
                 tc.tile_pool(name="ztps", bufs=2, space="PSUM") as ztps, \
                 tc.tile_pool(name="ops", bufs=2, space="PSUM") as ops, \
                 tc.tile_pool(name="att", bufs=6) as att, \
                 tc.tile_pool(name="osbp", bufs=2) as osbp, \
                 tc.tile_pool(name="nrm", bufs=2) as nrm:
                for j in range(NQC):
                    q0 = QC * j
                    zTn = nrm.tile([P, QC], F32R, tag="zTn")
                    nkb = 4 * j + 4
                    for h in range(2):
                        hs = slice(64 * h, 64 * (h + 1))
                        zt = ztps.tile([P, QC], F32, tag="zt")
                        for kb in range(nkb):
                            i = kb - 4 * j
                            qoff = 0
                            if i >= 0:
                                qoff = 256 if i == 3 else P * i
                            fr = QC - qoff
                            sc_ps = scps.tile([P, fr], F32, tag="sc")
                            nc.tensor.matmul(sc_ps[:],
                                             kT[hs, P * kb:P * (kb + 1)],
                                             qTn[hs, q0 + qoff:q0 + QC],
                                             start=True, stop=True)
                            if i == 3:
                                nc.vector.tensor_add(sc_ps[:, 0:256], sc_ps[:, 0:256], tri2[:])
                            elif i >= 0:
                                nc.vector.tensor_add(sc_ps[:, 0:P], sc_ps[:, 0:P], tri1[:])
                            eT = att.tile([P, fr], F32R, tag="eT")
                            nc.scalar.activation(eT[:], sc_ps[:], AF.Exp, bias=0.0,
                                                 scale=frec[:, 32 + 2 * kb + h:33 + 2 * kb + h])
                            if h == 0:
                                nc.tensor.matmul(zt[0:65, qoff:QC],
                                                 vaug0[:, 65 * kb:65 * kb + 65], eT[:],
                                                 start=(kb == 0), stop=(kb == nkb - 1))
                            else:
                                nc.tensor.matmul(zt[:, qoff:QC],
                                                 vaug1[:, P * kb:P * (kb + 1)], eT[:],
                                                 start=(kb == 0), stop=(kb == nkb - 1))
                        # softmax denominators: h0 row at p64, h1 row at p0
                        dp = 64 if h == 0 else 0
                        ds = slice(dp, dp + 1)
                        dra = nrm.tile([P, QC], F32, tag="dra")
                        drb = nrm.tile([P, QC], F32, tag="drb")
                        nc.vector.tensor_copy(dra[ds, :], zt[ds, :])
                        nc.vector.reciprocal(drb[ds, :], dra[ds, :])
                        if h == 0:
                            d0 = nrm.tile([1, QC], F32, tag="d0")
                            nc.sync.dma_start(d0[:], drb[64:65, :])
                            src_row = d0[:]
                        else:
                            src_row = drb[0:1, :]
                        db = nrm.tile([P, QC], F32, tag="db")
                        nc.gpsimd.partition_broadcast(db[:], src_row)
                        zrows = zt[0:64, :] if h == 0 else zt[64:128, :]
                        nc.vector.tensor_mul(zTn[hs, :], zrows, db[hs, :])
                    osb = osbp.tile([P, 4 * QC], F32, tag="osb")
                    for dc in range(NCH):
                        op = ops.tile([P, QC], F32, tag="op")
                        nc.tensor.matmul(op[:], woT[:, P * dc:P * (dc + 1)], zTn[:],
                                         start=True, stop=True)
                        if dc % 2 == 0:
                            nc.vector.tensor_copy(osb[:, QC * dc:QC * (dc + 1)], op[:])
                        else:
                            nc.scalar.copy(osb[:, QC * dc:QC * (dc + 1)], op[:])
                    nc.sync.dma_start(
                        out.rearrange("(c p) s -> p c s", p=P)[:, :, q0:q0 + QC],
                        osb[:].rearrange("p (c s) -> p c s", c=NCH))

    nc.finalize()
    return nc


def _pack_weights(Wqkv, Wo, h0):
    """Per-core packed/pretransposed weights [128, 2048] = wqT|wkT|wvT|woT."""
    r = slice(DH * h0, DH * h0 + P)
    parts = []
    for wslice in (Wqkv[r, :], Wqkv[D:][r, :], Wqkv[2 * D:][r, :]):
        parts.append(wslice.reshape(P, NCH, P).transpose(2, 1, 0).reshape(P, D))
    parts.append(Wo[:, r].T)
    return np.ascontiguousarray(np.concatenate(parts, axis=1), dtype=np.float32)


def _numpy_reference(x, Wqkv, Wo, scale_q, scale_k, mask):
    b, s, d = x.shape
    dh = d // NH
    qkv = x @ Wqkv.T
    q, k, v = np.split(qkv, 3, axis=-1)

    def rms(t, scale):
        r = np.sqrt(np.mean(np.square(t), axis=-1, keepdims=True)) + EPS
        return t / r * scale

    q = rms(q.reshape(b, s, NH, dh), scale_q)
    k = rms(k.reshape(b, s, NH, dh), scale_k)
    v = v.reshape(b, s, NH, dh)
    attn = np.einsum('bqhd,bkhd->bhqk', q, k)
    attn = np.where(mask[None, None], -np.inf, attn)
    attn = attn - attn.max(axis=-1, keepdims=True)
    p = np.exp(attn)
    p = p / p.sum(axis=-1, keepdims=True)
    z = np.einsum('bhqk,bkhd->bqhd', p, v).reshape(b, s, d)
    return (z @ Wo.T).astype(np.float32)


def kernel(x, Wqkv, Wo, scale_q, scale_k, mask):
    x = np.asarray(x, np.float32)
    Wqkv = np.asarray(Wqkv, np.float32)
    Wo = np.asarray(Wo, np.float32)
    causal = np.triu(np.ones((S, S), dtype=bool), k=1)
    if (not np.allclose(np.asarray(scale_q), 1.0) or
            not np.allclose(np.asarray(scale_k), 1.0) or
            not np.array_equal(np.asarray(mask), causal) or
            x.shape != (B, S, D)):
        return _numpy_reference(x, Wqkv, Wo, np.asarray(scale_q), np.asarray(scale_k),
                                np.asarray(mask))

    if "nc" not in _CACHE:
        _CACHE["nc"] = _build_program()
    nc = _CACHE["nc"]

    in_maps = []
    for core in range(8):
        bb = core // 4
        h0 = 2 * (core % 4)
        in_maps.append({
            "xb": np.ascontiguousarray(x[bb]),
            "wt4": _pack_weights(Wqkv, Wo, h0),
            "idn": np.eye(P, dtype=np.float32),
        })
    _CACHE["last_in_maps"] = in_maps
    res = run_bass_kernel_spmd(nc, in_maps, core_ids=list(range(8)))
    outp = np.zeros((B, S, D), np.float32)
    for core in range(8):
        outp[core // 4] += res.results[core]["out"].T
    return outp
